# revision 1
# baseline (speedup 1.0000x reference)
"""DBRX MoE experts kernel for Trainium2 (8 NeuronCores).

Strategy:
  - Router (logits -> softmax -> top-2 -> renormalize) computed on host in numpy
    (0.01% of FLOPs); it determines the token->expert dispatch, i.e. the sharding.
  - Tensor-parallel over the FFN intermediate dim across 8 cores: core c owns
    I-slice [c*512:(c+1)*512) of every expert (ws rows for gate and up, w2s cols).
  - Top-2 sparsity: tokens are packed per expert (padded to 256-token blocks);
    each core runs gate/up matmuls (contraction D=2048), SwiGLU, down matmul
    (contraction I_shard=512), scales rows by combine weights, and writes the
    packed rows contiguously.
  - Matmuls run in fp32r (11-bit mantissa, full PE rate at free dim >= 256).
    All weight/activation inputs are pre-rounded to fp32r on host (bit-exact
    with the device rounding); the on-chip h = silu(gate)*up write rounds to
    fp32r for free via the DVE output dtype.
  - A ReduceScatter over the 8 cores sums the I-shard partials of the packed
    rows; core c returns packed rows [c*npad/8:(c+1)*npad/8). The host
    concatenates the shards and assembles out[t] = packed[pos0[t]] +
    packed[pos1[t]] (the two expert contributions, already weighted on device).
"""

import math

import numpy as np

T = 4096
D = 2048
E = 8
I = 4096
TOPK = 2
NCORES = 8
ISH = I // NCORES  # 512, per-core I shard
BLK = 256  # token block (matmul free dim for gate/up)
P = 128
DCH = D // P  # 16 d-chunks
ICH = ISH // P  # 4 i-chunks


def _round_fp32r(x: np.ndarray) -> np.ndarray:
    """Round-to-nearest-even to 11 explicit mantissa bits (device-verified bit-exact)."""
    b = np.ascontiguousarray(x, dtype=np.float32).view(np.uint32).astype(np.uint64)
    bias = ((b >> 12) & 1) + np.uint64(0x7FF)
    r = ((b + bias) >> 12 << 12).astype(np.uint32)
    return r.view(np.float32)


def _host_router(x, router_w):
    """Replicate reference routing in numpy (fp32)."""
    logits = (x.astype(np.float64) @ router_w.astype(np.float64).T).astype(np.float32)
    m = logits.max(axis=-1, keepdims=True)
    ex = np.exp((logits - m).astype(np.float32))
    probs = ex / ex.sum(axis=-1, keepdims=True)
    # top-2, ties to lower index (matches jax.lax.top_k)
    top1 = probs.argmax(axis=-1)
    p = probs.copy()
    p[np.arange(T), top1] = -1.0
    top2 = p.argmax(axis=-1)
    w1 = probs[np.arange(T), top1]
    w2 = probs[np.arange(T), top2]
    s = w1 + w2
    return top1.astype(np.int64), top2.astype(np.int64), (w1 / s).astype(np.float32), (w2 / s).astype(np.float32)


_CACHE: dict = {}


def _build_bass(nblk: list[int], npad: int):
    """Build the 8-core SPMD Bass program. nblk[e] = number of 256-token blocks
    for expert e; npad = total packed (padded) tokens."""
    import concourse.bacc as bacc
    import concourse.mybir as mybir
    import concourse.tile as tile

    f32 = mybir.dt.float32
    f32r = mybir.dt.float32r
    nsub = npad // P  # 128-row subblocks

    nc = bacc.Bacc("TRN2", target_bir_lowering=False)

    nblk_tot = npad // BLK
    xtp_d = nc.dram_tensor("xtp", [P, nblk_tot, DCH, BLK], f32r, kind="ExternalInput")
    wst_d = nc.dram_tensor("wst", [E, DCH, P, 2 * ISH], f32r, kind="ExternalInput")
    w2st_d = nc.dram_tensor("w2st", [E, ICH, P, D], f32r, kind="ExternalInput")
    cw_d = nc.dram_tensor("cw", [P, nsub], f32, kind="ExternalInput")
    out_d = nc.dram_tensor("out", [npad // NCORES, D], f32, kind="ExternalOutput")

    with tile.TileContext(nc) as tc:
        with (
            tc.tile_pool(name="dram", bufs=1, space="DRAM") as dram_pool,
            tc.tile_pool(name="wpool", bufs=23) as wpool,
            tc.tile_pool(name="w2pool", bufs=5) as w2pool,
            tc.tile_pool(name="xpool", bufs=2) as xpool,
            tc.tile_pool(name="spool", bufs=3) as spool,
            tc.tile_pool(name="hpool", bufs=2) as hpool,
            tc.tile_pool(name="opool", bufs=2) as opool,
            tc.tile_pool(name="const", bufs=1) as const_pool,
            tc.tile_pool(name="ph", bufs=6, space="PSUM") as ph_pool,
            tc.tile_pool(name="po", bufs=2, space="PSUM") as po_pool,
        ):
            packed = dram_pool.tile([npad, D], f32)
            rs_out = dram_pool.tile([npad // NCORES, D], f32)

            # first token block issued before any weights so the first matmul's
            # deps (xt0 + wst tile 0) are at the head of the DMA queue
            xt0 = xpool.tile([P, DCH, BLK], f32r, tag="xt")
            nc.sync.dma_start(xt0[:], xtp_d[:, 0])

            # combine weights, resident (needed first at phase 3 of block 0)
            cw_sb = const_pool.tile([P, nsub], f32)
            nc.sync.dma_start(cw_sb[:], cw_d[:])

            gblk = 0
            for e in range(E):
                wst_tiles = []
                for dc in range(DCH):
                    wt = wpool.tile([P, 2 * ISH], f32r, tag="wst")
                    nc.sync.dma_start(wt[:], wst_d[e, dc])
                    wst_tiles.append(wt)
                w2_tiles = []
                for ic in range(ICH):
                    w2t = w2pool.tile([P, D], f32r, tag="w2st")
                    nc.sync.dma_start(w2t[:], w2st_d[e, ic])
                    w2_tiles.append(w2t)

                for _b in range(nblk[e]):
                    if gblk == 0:
                        xt = xt0
                    else:
                        xt = xpool.tile([P, DCH, BLK], f32r, tag="xt")
                        nc.sync.dma_start(xt[:], xtp_d[:, gblk])

                    # phase 1: gate/up in ic-pairs; each accumulation group gets
                    # its own PSUM bank (start=True clears the whole bank)
                    hT = hpool.tile([P, ICH, BLK], f32r, tag="hT")
                    for half in range(ICH // 2):
                        phg = [
                            ph_pool.tile([P, BLK], f32, tag="ph", name=f"phg_{gblk}_{half}_{j}")
                            for j in range(2)
                        ]
                        phu = [
                            ph_pool.tile([P, BLK], f32, tag="ph", name=f"phu_{gblk}_{half}_{j}")
                            for j in range(2)
                        ]
                        for dc in range(DCH):
                            wt = wst_tiles[dc]
                            for j in range(2):
                                ic = half * 2 + j
                                nc.tensor.matmul(
                                    phg[j][:],
                                    wt[:, ic * P : (ic + 1) * P],
                                    xt[:, dc, :],
                                    start=(dc == 0),
                                    stop=(dc == DCH - 1),
                                )
                                nc.tensor.matmul(
                                    phu[j][:],
                                    wt[:, ISH + ic * P : ISH + (ic + 1) * P],
                                    xt[:, dc, :],
                                    start=(dc == 0),
                                    stop=(dc == DCH - 1),
                                )
                        for j in range(2):
                            ic = half * 2 + j
                            sg = spool.tile([P, BLK], f32, tag="sg")
                            nc.scalar.activation(
                                sg[:], phg[j][:], mybir.ActivationFunctionType.Silu
                            )
                            nc.vector.tensor_mul(hT[:, ic, :], sg[:], phu[j][:])

                    # phase 3: down proj per 128-token subblock
                    for s in range(BLK // P):
                        gsub = gblk * (BLK // P) + s
                        osb = opool.tile([P, D], f32, tag="osb")
                        for dt_i in range(D // 512):
                            po_t = po_pool.tile([P, 512], f32, tag="po")
                            for ic in range(ICH):
                                nc.tensor.matmul(
                                    po_t[:],
                                    hT[:, ic, s * P : (s + 1) * P],
                                    w2_tiles[ic][:, dt_i * 512 : (dt_i + 1) * 512],
                                    start=(ic == 0),
                                    stop=(ic == ICH - 1),
                                )
                            # evacuate + scale by combine weight (split ACT/DVE)
                            if dt_i < 2:
                                nc.scalar.activation(
                                    osb[:, dt_i * 512 : (dt_i + 1) * 512],
                                    po_t[:],
                                    mybir.ActivationFunctionType.Copy,
                                    scale=cw_sb[:, gsub : gsub + 1],
                                )
                            else:
                                nc.vector.tensor_scalar_mul(
                                    osb[:, dt_i * 512 : (dt_i + 1) * 512],
                                    po_t[:],
                                    cw_sb[:, gsub : gsub + 1],
                                )
                        nc.sync.dma_start(
                            packed[gsub * P : (gsub + 1) * P, :], osb[:]
                        )
                    gblk += 1

                # expert e's packed rows are final on every core here; reduce-
                # scatter them now so the collective overlaps the next expert
                base = (gblk - nblk[e]) * BLK
                sz = nblk[e] * BLK
                nc.gpsimd.collective_compute(
                    "ReduceScatter",
                    mybir.AluOpType.add,
                    replica_groups=[list(range(NCORES))],
                    ins=[packed[base : base + sz].opt()],
                    outs=[rs_out[base // NCORES : (base + sz) // NCORES].opt()],
                )
            nc.sync.dma_start(out_d[:], rs_out[:])

    nc.compile()
    return nc


def _prepare(hidden_states, router_w, ws, w2s):
    """Host-side routing, packing, transposes, fp32r rounding. Returns
    (nblk, npad, pos, shared inputs dict, per-core weight arrays)."""
    x = np.asarray(hidden_states, dtype=np.float32).reshape(T, D)
    router_w = np.asarray(router_w, dtype=np.float32)
    ws = np.asarray(ws, dtype=np.float32)
    w2s = np.asarray(w2s, dtype=np.float32)

    top1, top2, w1, w2 = _host_router(x, router_w)

    # per-expert token lists and weights
    toks: list[list[int]] = [[] for _ in range(E)]
    cws: list[list[float]] = [[] for _ in range(E)]
    for ti, wi in [(top1, w1), (top2, w2)]:
        for t in range(T):
            e = int(ti[t])
            toks[e].append(t)
            cws[e].append(float(wi[t]))

    nblk = []
    perm = []
    cw = []
    # pos[k, t] = packed position of token t's k-th expert contribution
    pos = np.zeros((TOPK, T), dtype=np.int64)
    seen = np.zeros(T, dtype=np.int64)
    for e in range(E):
        n = len(toks[e])
        npd = math.ceil(n / BLK) * BLK if n > 0 else 0
        nblk.append(npd // BLK)
        base = len(perm)
        for j, t in enumerate(toks[e]):
            pos[seen[t], t] = base + j
            seen[t] += 1
        perm.extend(toks[e])
        cw.extend(cws[e])
        perm.extend([0] * (npd - n))
        cw.extend([0.0] * (npd - n))
    npad = len(perm)
    perm = np.asarray(perm, dtype=np.int64)

    # packed-transposed tokens, block-contiguous per partition:
    # xtp[p, b, dc, j] = x[perm[b*BLK + j], dc*128 + p]
    xr = _round_fp32r(x)
    nblk_tot = npad // BLK
    xtp = np.ascontiguousarray(
        xr[perm].reshape(nblk_tot, BLK, DCH, P).transpose(3, 0, 2, 1)
    )  # [P, nblk_tot, DCH, BLK]

    nsub = npad // P
    cw_a = np.asarray(cw, dtype=np.float32).reshape(nsub, P).T.copy()  # [P, nsub]

    # per-core weights
    wst_all = []
    w2st_all = []
    gate = ws[:, :I, :]  # [E, I, D]
    up = ws[:, I:, :]
    for c in range(NCORES):
        lo, hi = c * ISH, (c + 1) * ISH
        # [E, DCH, P, 2*ISH]: [.., d-part, gate(ISH)||up(ISH)]
        g = gate[:, lo:hi, :].reshape(E, ISH, DCH, P).transpose(0, 2, 3, 1)
        u = up[:, lo:hi, :].reshape(E, ISH, DCH, P).transpose(0, 2, 3, 1)
        wst = np.concatenate([g, u], axis=3)
        wst_all.append(_round_fp32r(np.ascontiguousarray(wst)))
        # w2s[e] is [D, I]; w2sT slice = w2s[:, :, lo:hi].T -> [E, ISH, D] -> [E, ICH, P, D]
        w2t = w2s[:, :, lo:hi].transpose(0, 2, 1).reshape(E, ICH, P, D)
        w2st_all.append(_round_fp32r(np.ascontiguousarray(w2t)))

    shared = {"xtp": xtp, "cw": cw_a}
    return nblk, npad, pos, shared, wst_all, w2st_all


def kernel(hidden_states, router_w, ws, w2s):
    from concourse import bass_utils

    hs = np.asarray(hidden_states)
    B, S, _ = hs.shape
    nblk, npad, pos, shared, wst_all, w2st_all = _prepare(hidden_states, router_w, ws, w2s)

    key = (tuple(nblk), npad)
    if key not in _CACHE:
        _CACHE[key] = _build_bass(nblk, npad)
    nc = _CACHE[key]

    in_maps = [
        {**shared, "wst": wst_all[c], "w2st": w2st_all[c]} for c in range(NCORES)
    ]
    res = bass_utils.run_bass_kernel_spmd(nc, in_maps, core_ids=list(range(NCORES)))
    # per-expert chunked RS: within each expert's row range, core c holds the
    # c-th eighth; reassemble the full packed array
    npad_total = sum(nblk) * BLK
    packed = np.empty((npad_total, D), dtype=np.float32)
    base = 0
    for e in range(E):
        sz = nblk[e] * BLK
        sz8 = sz // NCORES
        for c in range(NCORES):
            packed[base + c * sz8 : base + (c + 1) * sz8] = res.results[c]["out"][
                base // NCORES : base // NCORES + sz8
            ]
        base += sz
    out = packed[pos[0]] + packed[pos[1]]  # the two (device-weighted) expert contributions
    return out.reshape(B, S, D).astype(np.float32)



# revision 10
# speedup vs baseline: 1.5786x; 1.5786x over previous
"""DBRX MoE experts kernel for Trainium2 (8 NeuronCores).

Strategy (expert-parallel + fp8 DoubleRow):
  - Router (logits -> softmax -> top-2 -> renormalize) on host in numpy; it
    determines the token->expert dispatch.
  - Expert-parallel: core c owns expert c end-to-end (full FFN), processing
    the ~1030 tokens routed to it, padded to a uniform NB blocks of 384 so
    all 8 cores run the identical SPMD program. No collectives.
  - All matmuls run in fp8 e4m3 with MatmulPerfMode.DoubleRow (two k-rows
    per partition, 0.5 PE cycles per output row). Accuracy is recovered with
    a 3-term error-compensated product: for operands a ~ a_hi + a_lo and
    b ~ b_hi + b_lo (both split host- or device-side into two e4m3 levels at
    a shared power-of-two scale), a.b ~ a_hi.b_hi + a_lo.b_hi + a_hi.b_lo.
    Measured end-to-end rel err ~2e-3 (tolerance 2e-2).
  - Phase 1 (gate/up): per I-tile of 128 rows and 384-token block, psum
    accumulates 8 k-chunks x 3 terms of DoubleRow matmuls (x moving).
    ACT computes silu(gate); DVE computes h = silu(gate)*up scaled to fp8
    range, then h_hi = fp8(h), h_lo = fp8(h - h_hi).
  - Phase 2 (down): W2 moving in 512-wide D chunks, h stationary; psum
    [128 tokens, 512 D] accumulates 16 I-pairs x 3 terms; ACT evacuates with
    the per-token combine weight folded in; DMA straight to DRAM.
  - Host assembles out[t] = packed[e0][row0] + packed[e1][row1].
"""

import math

import numpy as np
import ml_dtypes

T = 4096
D = 2048
E = 8
I = 4096
NCORES = 8
BLKT = 384          # tokens per block (matmul moving free dim)
KCH = D // 256      # 8 k-chunks of 256 (DoubleRow pairs) for gate/up
ITILES = I // 128   # 32 I-tiles of 128 rows
IPAIR = I // 256    # 16 I-pair chunks for down proj
DCHK = D // 512     # 4 D-chunks of 512 for down proj

SX = 16.0           # x scale
SW = 1024.0         # W1 (gate/up) scale
SH = 8.0            # h scale
SW2 = 1024.0        # W2 scale

E4 = ml_dtypes.float8_e4m3

_CACHE: dict = {}


def _host_router(x, router_w):
    """Replicate reference routing in numpy (fp32)."""
    logits = (x.astype(np.float64) @ router_w.astype(np.float64).T).astype(np.float32)
    m = logits.max(axis=-1, keepdims=True)
    ex = np.exp((logits - m).astype(np.float32))
    probs = ex / ex.sum(axis=-1, keepdims=True)
    top1 = probs.argmax(axis=-1)
    p = probs.copy()
    p[np.arange(T), top1] = -1.0
    top2 = p.argmax(axis=-1)
    w1 = probs[np.arange(T), top1]
    w2 = probs[np.arange(T), top2]
    s = w1 + w2
    return top1.astype(np.int64), top2.astype(np.int64), (w1 / s).astype(np.float32), (w2 / s).astype(np.float32)


def _split_fp8(a, scale):
    """Two-level e4m3 split of a*scale: returns (hi, lo) fp8 arrays with
    a*scale ~ hi + lo."""
    s = (a * scale).astype(np.float32)
    hi = s.astype(E4)
    lo = (s - hi.astype(np.float32)).astype(E4)
    return hi, lo


def _build_bass(nb: int):
    """8-core SPMD program; nb = number of 384-token blocks per core."""
    import concourse.bacc as bacc
    import concourse.mybir as mybir
    import concourse.tile as tile

    f32 = mybir.dt.float32
    f8 = mybir.dt.float8e4
    DR = mybir.MatmulPerfMode.DoubleRow
    Silu = mybir.ActivationFunctionType.Silu
    Copy = mybir.ActivationFunctionType.Copy
    mul_op = mybir.AluOpType.mult
    sub_op = mybir.AluOpType.subtract

    ntok = nb * BLKT
    tsub = ntok // 128

    nc = bacc.Bacc("TRN2", target_bir_lowering=False)
    xh_d = nc.dram_tensor("xh", [128, KCH, 2, ntok], f8, kind="ExternalInput")
    xl_d = nc.dram_tensor("xl", [128, KCH, 2, ntok], f8, kind="ExternalInput")
    w1h_d = nc.dram_tensor("w1h", [128, ITILES, KCH, 2, 256], f8, kind="ExternalInput")
    w1l_d = nc.dram_tensor("w1l", [128, ITILES, KCH, 2, 256], f8, kind="ExternalInput")
    w2h_d = nc.dram_tensor("w2h", [128, DCHK, IPAIR, 2, 512], f8, kind="ExternalInput")
    w2l_d = nc.dram_tensor("w2l", [128, DCHK, IPAIR, 2, 512], f8, kind="ExternalInput")
    cw_d = nc.dram_tensor("cw", [128, tsub], f32, kind="ExternalInput")
    out_d = nc.dram_tensor("out", [ntok, D], f32, kind="ExternalOutput")

    IC = 1          # I-tiles per streamed W1 chunk
    NIC = ITILES // IC

    with tile.TileContext(nc) as tc:
        with (
            tc.tile_pool(name="xpool", bufs=1) as xpool,
            tc.tile_pool(name="hpool", bufs=1) as hpool,
            tc.tile_pool(name="wpool", bufs=2) as wpool,
            tc.tile_pool(name="sgpool", bufs=3) as sgpool,
            tc.tile_pool(name="hfpool", bufs=3) as hfpool,
            tc.tile_pool(name="evpool", bufs=3) as evpool,
            tc.tile_pool(name="const", bufs=1) as const_pool,
            tc.tile_pool(name="php", bufs=4, space="PSUM") as php,
            tc.tile_pool(name="pop", bufs=3, space="PSUM") as pop,
        ):
            # ---- initial DMAs: W1 chunk 0 (hi+lo), then x (hi+lo), cw ----
            w1_tiles = [None] * NIC  # (hi, lo) per chunk, allocated on demand
            w1h_t0 = wpool.tile([128, KCH, 2, 256], f8, tag="w1h")
            nc.sync.dma_start(w1h_t0[:], w1h_d[:, 0])
            w1l_t0 = wpool.tile([128, KCH, 2, 256], f8, tag="w1l")
            nc.sync.dma_start(w1l_t0[:], w1l_d[:, 0])
            w1_tiles[0] = (w1h_t0, w1l_t0)

            xh = xpool.tile([128, KCH, 2, ntok], f8, tag="xh")
            nc.sync.dma_start(xh[:], xh_d[:])
            xl = xpool.tile([128, KCH, 2, ntok], f8, tag="xl")
            nc.sync.dma_start(xl[:], xl_d[:])

            cw_sb = const_pool.tile([128, tsub], f32)
            nc.sync.dma_start(cw_sb[:], cw_d[:])

            hh = hpool.tile([128, ITILES, ntok], f8, tag="hh")
            hl = hpool.tile([128, ITILES, ntok], f8, tag="hl")

            # ---- phase 1: gate/up + h ----
            for ic in range(NIC):
                if w1_tiles[ic] is None:
                    w1h_t = wpool.tile([128, KCH, 2, 256], f8, tag="w1h")
                    nc.sync.dma_start(w1h_t[:], w1h_d[:, ic])
                    w1l_t = wpool.tile([128, KCH, 2, 256], f8, tag="w1l")
                    nc.sync.dma_start(w1l_t[:], w1l_d[:, ic])
                    w1_tiles[ic] = (w1h_t, w1l_t)
                w1h_t, w1l_t = w1_tiles[ic]
                for itl in range(IC):
                    it = ic
                    for b in range(nb):
                        ts0, ts1 = b * BLKT, (b + 1) * BLKT
                        pg = php.tile([128, 512], f32, tag="ph", name=f"pg_{it}_{b}")
                        pu = php.tile([128, 512], f32, tag="ph", name=f"pu_{it}_{b}")
                        # terms: (xh,w1h), (xh,w1l), (xl,w1h)
                        terms = [(xh, w1h_t), (xh, w1l_t), (xl, w1h_t)]
                        n_mm = KCH * len(terms)
                        i_mm = 0
                        for (xt, wt) in terms:
                            for k in range(KCH):
                                st = i_mm == 0
                                sp = i_mm == n_mm - 1
                                nc.tensor.matmul(
                                    pg[:, :BLKT],
                                    wt[:, k, :, 0:128],
                                    xt[:, k, :, ts0:ts1],
                                    start=st, stop=sp, perf_mode=DR,
                                )
                                nc.tensor.matmul(
                                    pu[:, :BLKT],
                                    wt[:, k, :, 128:256],
                                    xt[:, k, :, ts0:ts1],
                                    start=st, stop=sp, perf_mode=DR,
                                )
                                i_mm += 1
                        sg = sgpool.tile([128, BLKT], f32, tag="sg")
                        nc.scalar.activation(sg[:], pg[:, :BLKT], Silu, scale=1.0 / (SX * SW))
                        hf = hfpool.tile([128, BLKT], f32, tag="hf")
                        # hf = (pu * SH/(SX*SW)) * sg  == SH * h
                        nc.vector.scalar_tensor_tensor(
                            hf[:], pu[:, :BLKT], SH / (SX * SW), sg[:], mul_op, mul_op
                        )
                        nc.scalar.activation(hh[:, it, ts0:ts1], hf[:], Copy)
                        # h_lo = hf - h_hi
                        nc.vector.scalar_tensor_tensor(
                            hl[:, it, ts0:ts1], hf[:], 1.0, hh[:, it, ts0:ts1], mul_op, sub_op
                        )

            # ---- phase 2: down proj ----
            for dc in range(DCHK):
                w2h_t = wpool.tile([128, IPAIR, 2, 512], f8, tag="w2h")
                nc.sync.dma_start(w2h_t[:], w2h_d[:, dc])
                w2l_t = wpool.tile([128, IPAIR, 2, 512], f8, tag="w2l")
                nc.sync.dma_start(w2l_t[:], w2l_d[:, dc])
                for ts in range(tsub):
                    tt0, tt1 = ts * 128, (ts + 1) * 128
                    po = pop.tile([128, 512], f32, tag="po", name=f"po_{dc}_{ts}")
                    terms2 = [(hh, w2h_t), (hh, w2l_t), (hl, w2h_t)]
                    n_mm = IPAIR * len(terms2)
                    i_mm = 0
                    for (ht, wt) in terms2:
                        for q in range(IPAIR):
                            nc.tensor.matmul(
                                po[:],
                                ht[:, 2 * q:2 * q + 2, tt0:tt1],
                                wt[:, q],
                                start=(i_mm == 0), stop=(i_mm == n_mm - 1),
                                perf_mode=DR,
                            )
                            i_mm += 1
                    ev = evpool.tile([128, 512], f32, tag="ev")
                    nc.scalar.activation(ev[:], po[:], Copy, scale=cw_sb[:, ts:ts + 1])
                    nc.sync.dma_start(out_d[tt0:tt1, dc * 512:(dc + 1) * 512], ev[:])

    nc.compile()
    return nc


def _prepare(hidden_states, router_w, ws, w2s):
    x = np.asarray(hidden_states, dtype=np.float32).reshape(T, D)
    router_w = np.asarray(router_w, dtype=np.float32)
    ws = np.asarray(ws, dtype=np.float32)
    w2s = np.asarray(w2s, dtype=np.float32)

    top1, top2, w1, w2 = _host_router(x, router_w)

    toks: list[list[int]] = [[] for _ in range(E)]
    cws: list[list[float]] = [[] for _ in range(E)]
    for ti, wi in [(top1, w1), (top2, w2)]:
        for t in range(T):
            e = int(ti[t])
            toks[e].append(t)
            cws[e].append(float(wi[t]))

    max_n = max(len(tk) for tk in toks)
    nb = max(1, math.ceil(max_n / BLKT))
    ntok = nb * BLKT
    tsub = ntok // 128

    # pos[k, t] = row of token t's k-th contribution in its expert's output
    pos = np.zeros((2, T), dtype=np.int64)
    expert_of = np.zeros((2, T), dtype=np.int64)
    seen = np.zeros(T, dtype=np.int64)
    for e in range(E):
        for j, t in enumerate(toks[e]):
            pos[seen[t], t] = j
            expert_of[seen[t], t] = e
            seen[t] += 1

    in_maps = []
    for c in range(E):
        n = len(toks[c])
        perm = np.asarray(toks[c] + [0] * (ntok - n), dtype=np.int64)
        xe = x[perm]
        if n < ntok:
            xe[n:] = 0.0
        xhi, xlo = _split_fp8(xe, SX)  # [ntok, D]
        # [128, KCH, 2, ntok]: (p, k, j, t) = x[t, k*256 + j*128 + p]
        xh_a = np.ascontiguousarray(
            xhi.reshape(ntok, KCH, 2, 128).transpose(3, 1, 2, 0))
        xl_a = np.ascontiguousarray(
            xlo.reshape(ntok, KCH, 2, 128).transpose(3, 1, 2, 0))

        gate = ws[c, :I, :]   # [I, D]
        up = ws[c, I:, :]
        g_hi, g_lo = _split_fp8(gate, SW)
        u_hi, u_lo = _split_fp8(up, SW)

        def w1_layout(g, u):
            # [128, ITILES, KCH, 2, 256]: (p, it, k, j, m) =
            #   {gate,up}[it*128 + (m%128), k*256 + j*128 + p]
            g4 = g.reshape(ITILES, 128, KCH, 2, 128).transpose(4, 0, 2, 3, 1)
            u4 = u.reshape(ITILES, 128, KCH, 2, 128).transpose(4, 0, 2, 3, 1)
            return np.ascontiguousarray(np.concatenate([g4, u4], axis=4))

        w1h_a = w1_layout(g_hi, u_hi)
        w1l_a = w1_layout(g_lo, u_lo)

        w2T = w2s[c].T  # [I, D]
        w2_hi, w2_lo = _split_fp8(w2T, SW2)

        def w2_layout(w):
            # [128, DCHK, IPAIR, 2, 512]: (p, dc, q, j, d) =
            #   w2T[q*256 + j*128 + p, dc*512 + d]
            w4 = w.reshape(IPAIR, 2, 128, DCHK, 512).transpose(2, 3, 0, 1, 4)
            return np.ascontiguousarray(w4)

        w2h_a = w2_layout(w2_hi)
        w2l_a = w2_layout(w2_lo)

        cw = np.zeros(ntok, dtype=np.float32)
        cw[:n] = np.asarray(cws[c], dtype=np.float32)
        cw_a = np.ascontiguousarray(
            (cw / (SH * SW2)).reshape(tsub, 128).T)  # [128, tsub]

        in_maps.append({
            "xh": xh_a, "xl": xl_a,
            "w1h": w1h_a, "w1l": w1l_a,
            "w2h": w2h_a, "w2l": w2l_a,
            "cw": cw_a,
        })

    return nb, pos, expert_of, in_maps


def kernel(hidden_states, router_w, ws, w2s):
    from concourse import bass_utils

    hs = np.asarray(hidden_states)
    B, S, _ = hs.shape
    nb, pos, expert_of, in_maps = _prepare(hidden_states, router_w, ws, w2s)

    if nb not in _CACHE:
        _CACHE[nb] = _build_bass(nb)
    nc = _CACHE[nb]

    res = bass_utils.run_bass_kernel_spmd(nc, in_maps, core_ids=list(range(NCORES)))
    outs = [res.results[c]["out"] for c in range(NCORES)]  # [ntok, D] each

    out = np.zeros((T, D), dtype=np.float32)
    for k in range(2):
        e_arr = expert_of[k]
        p_arr = pos[k]
        for e in range(E):
            mask = e_arr == e
            out[mask] += outs[e][p_arr[mask]]
    return out.reshape(B, S, D).astype(np.float32)


# revision 13
# speedup vs baseline: 1.6796x; 1.0639x over previous
"""DBRX MoE experts kernel for Trainium2 (8 NeuronCores).

Strategy (expert-parallel + fp8 DoubleRow):
  - Router (logits -> softmax -> top-2 -> renormalize) on host in numpy; it
    determines the token->expert dispatch.
  - Expert-parallel: core c owns expert c end-to-end (full FFN), processing
    the ~1030 tokens routed to it, padded to a uniform NB blocks of 384 so
    all 8 cores run the identical SPMD program. No collectives.
  - All matmuls run in fp8 e4m3 with MatmulPerfMode.DoubleRow (two k-rows
    per partition, 0.5 PE cycles per output row). Accuracy is recovered with
    a 3-term error-compensated product: for operands a ~ a_hi + a_lo and
    b ~ b_hi + b_lo (both split host- or device-side into two e4m3 levels at
    a shared power-of-two scale), a.b ~ a_hi.b_hi + a_lo.b_hi + a_hi.b_lo.
    Measured end-to-end rel err ~2e-3 (tolerance 2e-2).
  - Phase 1 (gate/up): per I-tile of 128 rows and 384-token block, psum
    accumulates 8 k-chunks x 3 terms of DoubleRow matmuls (x moving).
    ACT computes silu(gate); DVE computes h = silu(gate)*up scaled to fp8
    range, then h_hi = fp8(h), h_lo = fp8(h - h_hi).
  - Phase 2 (down): W2 moving in 512-wide D chunks, h stationary; psum
    [128 tokens, 512 D] accumulates 16 I-pairs x 3 terms; ACT evacuates with
    the per-token combine weight folded in; DMA straight to DRAM.
  - Host assembles out[t] = packed[e0][row0] + packed[e1][row1].
"""

import math

import numpy as np
import ml_dtypes

T = 4096
D = 2048
E = 8
I = 4096
NCORES = 8
BLKT = 384          # tokens per block (matmul moving free dim)
KCH = D // 256      # 8 k-chunks of 256 (DoubleRow pairs) for gate/up
ITILES = I // 128   # 32 I-tiles of 128 rows
IPAIR = I // 256    # 16 I-pair chunks for down proj
DCHK = D // 512     # 4 D-chunks of 512 for down proj

SX = 16.0           # x scale
SW = 1024.0         # W1 (gate/up) scale
SH = 8.0            # h scale
SW2 = 1024.0        # W2 scale

E4 = ml_dtypes.float8_e4m3

_CACHE: dict = {}


def _host_router(x, router_w):
    """Replicate reference routing in numpy (fp32)."""
    logits = (x.astype(np.float64) @ router_w.astype(np.float64).T).astype(np.float32)
    m = logits.max(axis=-1, keepdims=True)
    ex = np.exp((logits - m).astype(np.float32))
    probs = ex / ex.sum(axis=-1, keepdims=True)
    top1 = probs.argmax(axis=-1)
    p = probs.copy()
    p[np.arange(T), top1] = -1.0
    top2 = p.argmax(axis=-1)
    w1 = probs[np.arange(T), top1]
    w2 = probs[np.arange(T), top2]
    s = w1 + w2
    return top1.astype(np.int64), top2.astype(np.int64), (w1 / s).astype(np.float32), (w2 / s).astype(np.float32)


def _split_fp8(a, scale):
    """Two-level e4m3 split of a*scale: returns (hi, lo) fp8 arrays with
    a*scale ~ hi + lo."""
    s = (a * scale).astype(np.float32)
    hi = s.astype(E4)
    lo = (s - hi.astype(np.float32)).astype(E4)
    return hi, lo


def _build_bass(sizes: tuple):
    """8-core SPMD program; sizes = per-core token block sizes (equal stride
    BMAX = sizes[0]; last may be shorter)."""
    import concourse.bacc as bacc
    import concourse.mybir as mybir
    import concourse.tile as tile

    f32 = mybir.dt.float32
    f8 = mybir.dt.float8e4
    DR = mybir.MatmulPerfMode.DoubleRow
    Silu = mybir.ActivationFunctionType.Silu
    Copy = mybir.ActivationFunctionType.Copy
    mul_op = mybir.AluOpType.mult
    sub_op = mybir.AluOpType.subtract

    nb = len(sizes)
    bmax = sizes[0]
    ncols = nb * bmax                # h column space (block b at b*bmax)
    tsub = -(-(ncols) // 128)        # ceil
    ntok = tsub * 128                # h/out row space

    nc = bacc.Bacc("TRN2", target_bir_lowering=False)
    xh_d = nc.dram_tensor("xh", [128, nb, KCH, 2, bmax], f8, kind="ExternalInput")
    xl_d = nc.dram_tensor("xl", [128, nb, KCH, 2, bmax], f8, kind="ExternalInput")
    w1h_d = nc.dram_tensor("w1h", [128, ITILES, KCH, 2, 256], f8, kind="ExternalInput")
    w1l_d = nc.dram_tensor("w1l", [128, ITILES, KCH, 2, 256], f8, kind="ExternalInput")
    w2h_d = nc.dram_tensor("w2h", [128, DCHK, IPAIR, 2, 512], f8, kind="ExternalInput")
    w2l_d = nc.dram_tensor("w2l", [128, DCHK, IPAIR, 2, 512], f8, kind="ExternalInput")
    cw_d = nc.dram_tensor("cw", [128, tsub], f32, kind="ExternalInput")
    out_d = nc.dram_tensor("out", [ntok, D], f32, kind="ExternalOutput")

    IC = 1          # I-tiles per streamed W1 chunk
    NIC = ITILES // IC

    with tile.TileContext(nc) as tc:
        with (
            tc.tile_pool(name="xpool", bufs=1) as xpool,
            tc.tile_pool(name="hpool", bufs=1) as hpool,
            tc.tile_pool(name="wpool", bufs=2) as wpool,
            tc.tile_pool(name="sgpool", bufs=3) as sgpool,
            tc.tile_pool(name="hfpool", bufs=3) as hfpool,
            tc.tile_pool(name="evpool", bufs=3) as evpool,
            tc.tile_pool(name="const", bufs=1) as const_pool,
            tc.tile_pool(name="php", bufs=4, space="PSUM") as php,
            tc.tile_pool(name="pop", bufs=3, space="PSUM") as pop,
        ):
            # ---- initial DMAs: W1 chunk 0 (hi+lo), then x (hi+lo), cw ----
            w1_tiles = [None] * NIC  # (hi, lo) per chunk, allocated on demand
            w1h_t0 = wpool.tile([128, KCH, 2, 256], f8, tag="w1h")
            nc.sync.dma_start(w1h_t0[:], w1h_d[:, 0])
            xh = xpool.tile([128, nb, KCH, 2, bmax], f8, tag="xh")
            nc.sync.dma_start(xh[:, 0], xh_d[:, 0])
            w1l_t0 = wpool.tile([128, KCH, 2, 256], f8, tag="w1l")
            nc.sync.dma_start(w1l_t0[:], w1l_d[:, 0])
            xl = xpool.tile([128, nb, KCH, 2, bmax], f8, tag="xl")
            nc.sync.dma_start(xl[:, 0], xl_d[:, 0])
            nc.sync.dma_start(xh[:, 1:], xh_d[:, 1:])
            nc.sync.dma_start(xl[:, 1:], xl_d[:, 1:])
            w1_tiles[0] = (w1h_t0, w1l_t0)

            cw_sb = const_pool.tile([128, tsub], f32)
            nc.sync.dma_start(cw_sb[:], cw_d[:])

            hh = hpool.tile([128, ITILES, ntok], f8, tag="hh")
            hl = hpool.tile([128, ITILES, ntok], f8, tag="hl")

            # ---- phase 1: gate/up + h ----
            for ic in range(NIC):
                if w1_tiles[ic] is None:
                    w1h_t = wpool.tile([128, KCH, 2, 256], f8, tag="w1h")
                    nc.sync.dma_start(w1h_t[:], w1h_d[:, ic])
                    w1l_t = wpool.tile([128, KCH, 2, 256], f8, tag="w1l")
                    nc.sync.dma_start(w1l_t[:], w1l_d[:, ic])
                    w1_tiles[ic] = (w1h_t, w1l_t)
                w1h_t, w1l_t = w1_tiles[ic]
                for itl in range(IC):
                    it = ic
                    for b in range(nb):
                        sb = sizes[b]
                        hc0 = b * bmax
                        pg = php.tile([128, 512], f32, tag="ph", name=f"pg_{it}_{b}")
                        pu = php.tile([128, 512], f32, tag="ph", name=f"pu_{it}_{b}")
                        # terms: (xh,w1h), (xh,w1l), (xl,w1h)
                        terms = [(xh, w1h_t), (xh, w1l_t), (xl, w1h_t)]
                        n_mm = KCH * len(terms)
                        i_mm = 0
                        for (xt, wt) in terms:
                            for k in range(KCH):
                                st = i_mm == 0
                                sp = i_mm == n_mm - 1
                                nc.tensor.matmul(
                                    pg[:, :sb],
                                    wt[:, k, :, 0:128],
                                    xt[:, b, k, :, 0:sb],
                                    start=st, stop=sp, perf_mode=DR,
                                )
                                nc.tensor.matmul(
                                    pu[:, :sb],
                                    wt[:, k, :, 128:256],
                                    xt[:, b, k, :, 0:sb],
                                    start=st, stop=sp, perf_mode=DR,
                                )
                                i_mm += 1
                        sg = sgpool.tile([128, bmax], f32, tag="sg")
                        nc.scalar.activation(sg[:, :sb], pg[:, :sb], Silu, scale=1.0 / (SX * SW))
                        hf = hfpool.tile([128, bmax], f32, tag="hf")
                        # hf = (pu * SH/(SX*SW)) * sg  == SH * h
                        nc.vector.scalar_tensor_tensor(
                            hf[:, :sb], pu[:, :sb], SH / (SX * SW), sg[:, :sb], mul_op, mul_op
                        )
                        nc.scalar.activation(hh[:, it, hc0:hc0 + sb], hf[:, :sb], Copy)
                        # h_lo = hf - h_hi
                        nc.vector.scalar_tensor_tensor(
                            hl[:, it, hc0:hc0 + sb], hf[:, :sb], 1.0,
                            hh[:, it, hc0:hc0 + sb], mul_op, sub_op
                        )

            # ---- phase 2: down proj ----
            for dc in range(DCHK):
                w2h_t = wpool.tile([128, IPAIR, 2, 512], f8, tag="w2h")
                nc.sync.dma_start(w2h_t[:], w2h_d[:, dc])
                w2l_t = wpool.tile([128, IPAIR, 2, 512], f8, tag="w2l")
                nc.sync.dma_start(w2l_t[:], w2l_d[:, dc])
                for ts in range(tsub):
                    tt0, tt1 = ts * 128, (ts + 1) * 128
                    po = pop.tile([128, 512], f32, tag="po", name=f"po_{dc}_{ts}")
                    terms2 = [(hh, w2h_t), (hh, w2l_t), (hl, w2h_t)]
                    n_mm = IPAIR * len(terms2)
                    i_mm = 0
                    for (ht, wt) in terms2:
                        for q in range(IPAIR):
                            nc.tensor.matmul(
                                po[:],
                                ht[:, 2 * q:2 * q + 2, tt0:tt1],
                                wt[:, q],
                                start=(i_mm == 0), stop=(i_mm == n_mm - 1),
                                perf_mode=DR,
                            )
                            i_mm += 1
                    ev = evpool.tile([128, 512], f32, tag="ev")
                    nc.scalar.activation(ev[:], po[:], Copy, scale=cw_sb[:, ts:ts + 1])
                    nc.sync.dma_start(out_d[tt0:tt1, dc * 512:(dc + 1) * 512], ev[:])

    nc.compile()
    return nc


def _prepare(hidden_states, router_w, ws, w2s):
    x = np.asarray(hidden_states, dtype=np.float32).reshape(T, D)
    router_w = np.asarray(router_w, dtype=np.float32)
    ws = np.asarray(ws, dtype=np.float32)
    w2s = np.asarray(w2s, dtype=np.float32)

    top1, top2, w1, w2 = _host_router(x, router_w)

    toks: list[list[int]] = [[] for _ in range(E)]
    cws: list[list[float]] = [[] for _ in range(E)]
    for ti, wi in [(top1, w1), (top2, w2)]:
        for t in range(T):
            e = int(ti[t])
            toks[e].append(t)
            cws[e].append(float(wi[t]))

    max_n = max(len(tk) for tk in toks)
    nb = max(1, math.ceil(max_n / BLKT))
    bmax = math.ceil(max_n / nb)          # equal block sizes
    sizes = tuple([bmax] * (nb - 1) + [max_n - bmax * (nb - 1)])
    ncols = nb * bmax
    tsub = math.ceil(ncols / 128)
    ntok = tsub * 128

    # pos[k, t] = row of token t's k-th contribution in its expert's output
    pos = np.zeros((2, T), dtype=np.int64)
    expert_of = np.zeros((2, T), dtype=np.int64)
    seen = np.zeros(T, dtype=np.int64)
    for e in range(E):
        for j, t in enumerate(toks[e]):
            pos[seen[t], t] = j
            expert_of[seen[t], t] = e
            seen[t] += 1

    in_maps = []
    nxpad = nb * bmax
    for c in range(E):
        n = len(toks[c])
        perm = np.asarray(toks[c] + [0] * (nxpad - n), dtype=np.int64)
        xe = x[perm]
        if n < nxpad:
            xe[n:] = 0.0
        xhi, xlo = _split_fp8(xe, SX)  # [nxpad, D]
        # [128, nb, KCH, 2, bmax]: (p, b, k, j, t) = x[b*bmax + t, k*256 + j*128 + p]
        xh_a = np.ascontiguousarray(
            xhi.reshape(nb, bmax, KCH, 2, 128).transpose(4, 0, 2, 3, 1))
        xl_a = np.ascontiguousarray(
            xlo.reshape(nb, bmax, KCH, 2, 128).transpose(4, 0, 2, 3, 1))

        gate = ws[c, :I, :]   # [I, D]
        up = ws[c, I:, :]
        g_hi, g_lo = _split_fp8(gate, SW)
        u_hi, u_lo = _split_fp8(up, SW)

        def w1_layout(g, u):
            # [128, ITILES, KCH, 2, 256]: (p, it, k, j, m) =
            #   {gate,up}[it*128 + (m%128), k*256 + j*128 + p]
            g4 = g.reshape(ITILES, 128, KCH, 2, 128).transpose(4, 0, 2, 3, 1)
            u4 = u.reshape(ITILES, 128, KCH, 2, 128).transpose(4, 0, 2, 3, 1)
            return np.ascontiguousarray(np.concatenate([g4, u4], axis=4))

        w1h_a = w1_layout(g_hi, u_hi)
        w1l_a = w1_layout(g_lo, u_lo)

        w2T = w2s[c].T  # [I, D]
        w2_hi, w2_lo = _split_fp8(w2T, SW2)

        def w2_layout(w):
            # [128, DCHK, IPAIR, 2, 512]: (p, dc, q, j, d) =
            #   w2T[q*256 + j*128 + p, dc*512 + d]
            w4 = w.reshape(IPAIR, 2, 128, DCHK, 512).transpose(2, 3, 0, 1, 4)
            return np.ascontiguousarray(w4)

        w2h_a = w2_layout(w2_hi)
        w2l_a = w2_layout(w2_lo)

        cw = np.zeros(ntok, dtype=np.float32)
        cw[:n] = np.asarray(cws[c], dtype=np.float32)
        cw_a = np.ascontiguousarray(
            (cw / (SH * SW2)).reshape(tsub, 128).T)  # [128, tsub]

        in_maps.append({
            "xh": xh_a, "xl": xl_a,
            "w1h": w1h_a, "w1l": w1l_a,
            "w2h": w2h_a, "w2l": w2l_a,
            "cw": cw_a,
        })

    return sizes, pos, expert_of, in_maps


def kernel(hidden_states, router_w, ws, w2s):
    from concourse import bass_utils

    hs = np.asarray(hidden_states)
    B, S, _ = hs.shape
    sizes, pos, expert_of, in_maps = _prepare(hidden_states, router_w, ws, w2s)

    if sizes not in _CACHE:
        _CACHE[sizes] = _build_bass(sizes)
    nc = _CACHE[sizes]

    res = bass_utils.run_bass_kernel_spmd(nc, in_maps, core_ids=list(range(NCORES)))
    outs = [res.results[c]["out"] for c in range(NCORES)]  # [ntok, D] each

    out = np.zeros((T, D), dtype=np.float32)
    for k in range(2):
        e_arr = expert_of[k]
        p_arr = pos[k]
        for e in range(E):
            mask = e_arr == e
            out[mask] += outs[e][p_arr[mask]]
    return out.reshape(B, S, D).astype(np.float32)


# revision 15
# speedup vs baseline: 1.6835x; 1.0023x over previous
"""DBRX MoE experts kernel for Trainium2 (8 NeuronCores).

Strategy (expert-parallel + fp8 DoubleRow):
  - Router (logits -> softmax -> top-2 -> renormalize) on host in numpy; it
    determines the token->expert dispatch.
  - Expert-parallel: core c owns expert c end-to-end (full FFN), processing
    the ~1030 tokens routed to it, padded to a uniform NB blocks of 384 so
    all 8 cores run the identical SPMD program. No collectives.
  - All matmuls run in fp8 e4m3 with MatmulPerfMode.DoubleRow (two k-rows
    per partition, 0.5 PE cycles per output row). Accuracy is recovered with
    a 3-term error-compensated product: for operands a ~ a_hi + a_lo and
    b ~ b_hi + b_lo (both split host- or device-side into two e4m3 levels at
    a shared power-of-two scale), a.b ~ a_hi.b_hi + a_lo.b_hi + a_hi.b_lo.
    Measured end-to-end rel err ~2e-3 (tolerance 2e-2).
  - Phase 1 (gate/up): per I-tile of 128 rows and 384-token block, psum
    accumulates 8 k-chunks x 3 terms of DoubleRow matmuls (x moving).
    ACT computes silu(gate); DVE computes h = silu(gate)*up scaled to fp8
    range, then h_hi = fp8(h), h_lo = fp8(h - h_hi).
  - Phase 2 (down): W2 moving in 512-wide D chunks, h stationary; psum
    [128 tokens, 512 D] accumulates 16 I-pairs x 3 terms; ACT evacuates with
    the per-token combine weight folded in; DMA straight to DRAM.
  - Host assembles out[t] = packed[e0][row0] + packed[e1][row1].
"""

import math

import numpy as np
import ml_dtypes

T = 4096
D = 2048
E = 8
I = 4096
NCORES = 8
BLKT = 384          # tokens per block (matmul moving free dim)
KCH = D // 256      # 8 k-chunks of 256 (DoubleRow pairs) for gate/up
ITILES = I // 128   # 32 I-tiles of 128 rows
IPAIR = I // 256    # 16 I-pair chunks for down proj
DCHK = D // 512     # 4 D-chunks of 512 for down proj

SX = 16.0           # x scale
SW = 1024.0         # W1 (gate/up) scale
SH = 8.0            # h scale
SW2 = 1024.0        # W2 scale

E4 = ml_dtypes.float8_e4m3

_CACHE: dict = {}


def _host_router(x, router_w):
    """Replicate reference routing in numpy (fp32)."""
    logits = (x.astype(np.float64) @ router_w.astype(np.float64).T).astype(np.float32)
    m = logits.max(axis=-1, keepdims=True)
    ex = np.exp((logits - m).astype(np.float32))
    probs = ex / ex.sum(axis=-1, keepdims=True)
    top1 = probs.argmax(axis=-1)
    p = probs.copy()
    p[np.arange(T), top1] = -1.0
    top2 = p.argmax(axis=-1)
    w1 = probs[np.arange(T), top1]
    w2 = probs[np.arange(T), top2]
    s = w1 + w2
    return top1.astype(np.int64), top2.astype(np.int64), (w1 / s).astype(np.float32), (w2 / s).astype(np.float32)


def _split_fp8(a, scale):
    """Two-level e4m3 split of a*scale: returns (hi, lo) fp8 arrays with
    a*scale ~ hi + lo."""
    s = (a * scale).astype(np.float32)
    hi = s.astype(E4)
    lo = (s - hi.astype(np.float32)).astype(E4)
    return hi, lo


def _build_bass(sizes: tuple):
    """8-core SPMD program; sizes = per-core token block sizes (equal stride
    BMAX = sizes[0]; last may be shorter)."""
    import concourse.bacc as bacc
    import concourse.mybir as mybir
    import concourse.tile as tile

    f32 = mybir.dt.float32
    f8 = mybir.dt.float8e4
    DR = mybir.MatmulPerfMode.DoubleRow
    Silu = mybir.ActivationFunctionType.Silu
    Copy = mybir.ActivationFunctionType.Copy
    mul_op = mybir.AluOpType.mult
    sub_op = mybir.AluOpType.subtract

    nb = len(sizes)
    bmax = sizes[0]
    ncols = nb * bmax                # h column space (block b at b*bmax)
    tsub = -(-(ncols) // 128)        # ceil
    ntok = tsub * 128                # h/out row space

    nc = bacc.Bacc("TRN2", target_bir_lowering=False)
    xh_d = nc.dram_tensor("xh", [128, nb, KCH, 2, bmax], f8, kind="ExternalInput")
    xl_d = nc.dram_tensor("xl", [128, nb, KCH, 2, bmax], f8, kind="ExternalInput")
    w1h_d = nc.dram_tensor("w1h", [128, ITILES, KCH, 2, 256], f8, kind="ExternalInput")
    w1l_d = nc.dram_tensor("w1l", [128, ITILES, KCH, 2, 256], f8, kind="ExternalInput")
    w2h_d = nc.dram_tensor("w2h", [128, DCHK, IPAIR, 2, 512], f8, kind="ExternalInput")
    w2l_d = nc.dram_tensor("w2l", [128, DCHK, IPAIR, 2, 512], f8, kind="ExternalInput")
    cw_d = nc.dram_tensor("cw", [128, tsub], f32, kind="ExternalInput")
    out_d = nc.dram_tensor("out", [ntok, D], f32, kind="ExternalOutput")

    IC = 1          # I-tiles per streamed W1 chunk
    NIC = ITILES // IC

    with tile.TileContext(nc) as tc:
        with (
            tc.tile_pool(name="xpool", bufs=1) as xpool,
            tc.tile_pool(name="hpool", bufs=1) as hpool,
            tc.tile_pool(name="wpool", bufs=2) as wpool,
            tc.tile_pool(name="sgpool", bufs=3) as sgpool,
            tc.tile_pool(name="hfpool", bufs=3) as hfpool,
            tc.tile_pool(name="evpool", bufs=3) as evpool,
            tc.tile_pool(name="const", bufs=1) as const_pool,
            tc.tile_pool(name="php", bufs=4, space="PSUM") as php,
            tc.tile_pool(name="pop", bufs=3, space="PSUM") as pop,
        ):
            # ---- initial DMAs: W1 chunk 0 (hi+lo), then x (hi+lo), cw ----
            w1_tiles = [None] * NIC  # (hi, lo) per chunk, allocated on demand
            w1h_t0 = wpool.tile([128, KCH, 2, 256], f8, tag="w1h")
            nc.sync.dma_start(w1h_t0[:], w1h_d[:, 0])
            xh_b = [
                xpool.tile([128, KCH, 2, bmax], f8, tag=f"xh{b}", name=f"xh{b}")
                for b in range(nb)
            ]
            xl_b = [
                xpool.tile([128, KCH, 2, bmax], f8, tag=f"xl{b}", name=f"xl{b}")
                for b in range(nb)
            ]
            nc.sync.dma_start(xh_b[0][:], xh_d[:, 0])
            w1l_t0 = wpool.tile([128, KCH, 2, 256], f8, tag="w1l")
            nc.sync.dma_start(w1l_t0[:], w1l_d[:, 0])
            nc.sync.dma_start(xl_b[0][:], xl_d[:, 0])
            for b in range(1, nb):
                nc.sync.dma_start(xh_b[b][:], xh_d[:, b])
                nc.sync.dma_start(xl_b[b][:], xl_d[:, b])
            w1_tiles[0] = (w1h_t0, w1l_t0)

            cw_sb = const_pool.tile([128, tsub], f32)
            nc.sync.dma_start(cw_sb[:], cw_d[:])

            hh = hpool.tile([128, ITILES, ntok], f8, tag="hh")
            hl = hpool.tile([128, ITILES, ntok], f8, tag="hl")

            # ---- phase 1: gate/up + h ----
            for ic in range(NIC):
                if w1_tiles[ic] is None:
                    w1h_t = wpool.tile([128, KCH, 2, 256], f8, tag="w1h")
                    nc.sync.dma_start(w1h_t[:], w1h_d[:, ic])
                    w1l_t = wpool.tile([128, KCH, 2, 256], f8, tag="w1l")
                    nc.sync.dma_start(w1l_t[:], w1l_d[:, ic])
                    w1_tiles[ic] = (w1h_t, w1l_t)
                w1h_t, w1l_t = w1_tiles[ic]
                for itl in range(IC):
                    it = ic
                    for b in range(nb):
                        sb = sizes[b]
                        hc0 = b * bmax
                        pg = php.tile([128, 512], f32, tag="ph", name=f"pg_{it}_{b}")
                        pu = php.tile([128, 512], f32, tag="ph", name=f"pu_{it}_{b}")
                        # terms: (xh,w1h), (xh,w1l), (xl,w1h)
                        terms = [(xh_b[b], w1h_t), (xh_b[b], w1l_t), (xl_b[b], w1h_t)]
                        n_mm = KCH * len(terms)
                        i_mm = 0
                        for (xt, wt) in terms:
                            for k in range(KCH):
                                st = i_mm == 0
                                sp = i_mm == n_mm - 1
                                nc.tensor.matmul(
                                    pg[:, :sb],
                                    wt[:, k, :, 0:128],
                                    xt[:, k, :, 0:sb],
                                    start=st, stop=sp, perf_mode=DR,
                                )
                                nc.tensor.matmul(
                                    pu[:, :sb],
                                    wt[:, k, :, 128:256],
                                    xt[:, k, :, 0:sb],
                                    start=st, stop=sp, perf_mode=DR,
                                )
                                i_mm += 1
                        sg = sgpool.tile([128, bmax], f32, tag="sg")
                        nc.scalar.activation(sg[:, :sb], pg[:, :sb], Silu, scale=1.0 / (SX * SW))
                        hf = hfpool.tile([128, bmax], f32, tag="hf")
                        # hf = (pu * SH/(SX*SW)) * sg  == SH * h
                        nc.vector.scalar_tensor_tensor(
                            hf[:, :sb], pu[:, :sb], SH / (SX * SW), sg[:, :sb], mul_op, mul_op
                        )
                        nc.scalar.activation(hh[:, it, hc0:hc0 + sb], hf[:, :sb], Copy)
                        # h_lo = hf - h_hi
                        nc.vector.scalar_tensor_tensor(
                            hl[:, it, hc0:hc0 + sb], hf[:, :sb], 1.0,
                            hh[:, it, hc0:hc0 + sb], mul_op, sub_op
                        )

            # ---- phase 2: down proj ----
            for dc in range(DCHK):
                w2h_t = wpool.tile([128, IPAIR, 2, 512], f8, tag="w2h")
                nc.sync.dma_start(w2h_t[:], w2h_d[:, dc])
                w2l_t = wpool.tile([128, IPAIR, 2, 512], f8, tag="w2l")
                nc.sync.dma_start(w2l_t[:], w2l_d[:, dc])
                for ts in range(tsub):
                    tt0, tt1 = ts * 128, (ts + 1) * 128
                    po = pop.tile([128, 512], f32, tag="po", name=f"po_{dc}_{ts}")
                    terms2 = [(hh, w2h_t), (hh, w2l_t), (hl, w2h_t)]
                    n_mm = IPAIR * len(terms2)
                    i_mm = 0
                    for (ht, wt) in terms2:
                        for q in range(IPAIR):
                            nc.tensor.matmul(
                                po[:],
                                ht[:, 2 * q:2 * q + 2, tt0:tt1],
                                wt[:, q],
                                start=(i_mm == 0), stop=(i_mm == n_mm - 1),
                                perf_mode=DR,
                            )
                            i_mm += 1
                    ev = evpool.tile([128, 512], f32, tag="ev")
                    nc.scalar.activation(ev[:], po[:], Copy, scale=cw_sb[:, ts:ts + 1])
                    nc.sync.dma_start(out_d[tt0:tt1, dc * 512:(dc + 1) * 512], ev[:])

    nc.compile()
    return nc


def _prepare(hidden_states, router_w, ws, w2s):
    x = np.asarray(hidden_states, dtype=np.float32).reshape(T, D)
    router_w = np.asarray(router_w, dtype=np.float32)
    ws = np.asarray(ws, dtype=np.float32)
    w2s = np.asarray(w2s, dtype=np.float32)

    top1, top2, w1, w2 = _host_router(x, router_w)

    toks: list[list[int]] = [[] for _ in range(E)]
    cws: list[list[float]] = [[] for _ in range(E)]
    for ti, wi in [(top1, w1), (top2, w2)]:
        for t in range(T):
            e = int(ti[t])
            toks[e].append(t)
            cws[e].append(float(wi[t]))

    max_n = max(len(tk) for tk in toks)
    nb = max(1, math.ceil(max_n / BLKT))
    bmax = math.ceil(max_n / nb)          # equal block sizes
    sizes = tuple([bmax] * (nb - 1) + [max_n - bmax * (nb - 1)])
    ncols = nb * bmax
    tsub = math.ceil(ncols / 128)
    ntok = tsub * 128

    # pos[k, t] = row of token t's k-th contribution in its expert's output
    pos = np.zeros((2, T), dtype=np.int64)
    expert_of = np.zeros((2, T), dtype=np.int64)
    seen = np.zeros(T, dtype=np.int64)
    for e in range(E):
        for j, t in enumerate(toks[e]):
            pos[seen[t], t] = j
            expert_of[seen[t], t] = e
            seen[t] += 1

    in_maps = []
    nxpad = nb * bmax
    for c in range(E):
        n = len(toks[c])
        perm = np.asarray(toks[c] + [0] * (nxpad - n), dtype=np.int64)
        xe = x[perm]
        if n < nxpad:
            xe[n:] = 0.0
        xhi, xlo = _split_fp8(xe, SX)  # [nxpad, D]
        # [128, nb, KCH, 2, bmax]: (p, b, k, j, t) = x[b*bmax + t, k*256 + j*128 + p]
        xh_a = np.ascontiguousarray(
            xhi.reshape(nb, bmax, KCH, 2, 128).transpose(4, 0, 2, 3, 1))
        xl_a = np.ascontiguousarray(
            xlo.reshape(nb, bmax, KCH, 2, 128).transpose(4, 0, 2, 3, 1))

        gate = ws[c, :I, :]   # [I, D]
        up = ws[c, I:, :]
        g_hi, g_lo = _split_fp8(gate, SW)
        u_hi, u_lo = _split_fp8(up, SW)

        def w1_layout(g, u):
            # [128, ITILES, KCH, 2, 256]: (p, it, k, j, m) =
            #   {gate,up}[it*128 + (m%128), k*256 + j*128 + p]
            g4 = g.reshape(ITILES, 128, KCH, 2, 128).transpose(4, 0, 2, 3, 1)
            u4 = u.reshape(ITILES, 128, KCH, 2, 128).transpose(4, 0, 2, 3, 1)
            return np.ascontiguousarray(np.concatenate([g4, u4], axis=4))

        w1h_a = w1_layout(g_hi, u_hi)
        w1l_a = w1_layout(g_lo, u_lo)

        w2T = w2s[c].T  # [I, D]
        w2_hi, w2_lo = _split_fp8(w2T, SW2)

        def w2_layout(w):
            # [128, DCHK, IPAIR, 2, 512]: (p, dc, q, j, d) =
            #   w2T[q*256 + j*128 + p, dc*512 + d]
            w4 = w.reshape(IPAIR, 2, 128, DCHK, 512).transpose(2, 3, 0, 1, 4)
            return np.ascontiguousarray(w4)

        w2h_a = w2_layout(w2_hi)
        w2l_a = w2_layout(w2_lo)

        cw = np.zeros(ntok, dtype=np.float32)
        cw[:n] = np.asarray(cws[c], dtype=np.float32)
        cw_a = np.ascontiguousarray(
            (cw / (SH * SW2)).reshape(tsub, 128).T)  # [128, tsub]

        in_maps.append({
            "xh": xh_a, "xl": xl_a,
            "w1h": w1h_a, "w1l": w1l_a,
            "w2h": w2h_a, "w2l": w2l_a,
            "cw": cw_a,
        })

    return sizes, pos, expert_of, in_maps


def kernel(hidden_states, router_w, ws, w2s):
    from concourse import bass_utils

    hs = np.asarray(hidden_states)
    B, S, _ = hs.shape
    sizes, pos, expert_of, in_maps = _prepare(hidden_states, router_w, ws, w2s)

    if sizes not in _CACHE:
        _CACHE[sizes] = _build_bass(sizes)
    nc = _CACHE[sizes]

    res = bass_utils.run_bass_kernel_spmd(nc, in_maps, core_ids=list(range(NCORES)))
    outs = [res.results[c]["out"] for c in range(NCORES)]  # [ntok, D] each

    out = np.zeros((T, D), dtype=np.float32)
    for k in range(2):
        e_arr = expert_of[k]
        p_arr = pos[k]
        for e in range(E):
            mask = e_arr == e
            out[mask] += outs[e][p_arr[mask]]
    return out.reshape(B, S, D).astype(np.float32)
